# revision 1
# baseline (speedup 1.0000x reference)
"""CTC loss (nn_CTCLossLayer) on 8 TRN2 NeuronCores, data-parallel over batch.

Algorithm: linear-space CTC forward recursion with per-chunk rescaling.
  - extended states S=2L+1=513, padded to 544 = 17 chunks x 32, laid out
    [68 partitions = 4 examples x 17 chunks, 32 free] per core (4 examples).
  - weights w[t,s] = kappa * exp(y_adj[t, ext_s]) built by one-hot matmuls
    (gather on the TensorEngine); log Z_t accumulated separately so no
    normalization is needed in the recursion.
  - per-step: boundary shift via a static stationary matmul (chunk k-1 tail
    -> chunk k head, scaled by r = exp(min(lnR[k-1]-lnR[k], 30))), then
    (a + a<<1 + skip * a<<2) * w[t] on the vector engine.
  - every 8 steps each chunk row renorms by max(rowmax, 1); log factors
    accumulate in lnR[p]; boundary ratio r recomputed.
  - finish: one pair-sum step, masked extraction of alpha[2len]+alpha[2len-1],
    log, + lnR - sum_t lnZ_t - T*ln(kappa).
Host only shards inputs / builds small index masks and means the 32 losses.
"""

import numpy as np
import ml_dtypes

import concourse.bass as bass
import concourse.bacc as bacc
import concourse.mybir as mybir
from concourse.bass_utils import run_bass_kernel_spmd
from concourse.tile import TileContext

F32 = mybir.dt.float32
BF16 = mybir.dt.bfloat16
AF = mybir.ActivationFunctionType
ALU = mybir.AluOpType
AX = mybir.AxisListType

B, T, V, L = 32, 2048, 128, 256
NEX = 4            # examples per core
NCORE = 8
S = 2 * L + 1      # 513
NCH, FW = 17, 32   # chunks per example, states per chunk (SP = 544)
SP = NCH * FW
NP = NEX * NCH     # 68 used partitions
K_RENORM = 8
CLIP = 20.0
KAPPA = float(np.float32(ml_dtypes.bfloat16(np.exp(5.35))))
LNKAP = float(np.log(np.float64(KAPPA)))
TTILE = 128        # t-tile for phase A
TCH = 512          # t-chunk for gather matmuls
NTB = T // TTILE   # 16 w-stream blocks

BLANK = 0
PEN = np.zeros(V, np.float32)
PEN[0] = 1.0
PEN[3] = 1.0
for _v in (11, 15, 19, 25, 31):
    PEN[_v] = 5.0

_CACHED = {}


def _build_program():
    if "nc" in _CACHED:
        return _CACHED["nc"]
    nc = bacc.Bacc("TRN2", target_bir_lowering=False, debug=False,
                   num_devices=NCORE)
    y = nc.dram_tensor("y", [NEX, T, V], F32, kind="ExternalInput").ap()
    oh = nc.dram_tensor("oh", [NEX, 5, V, 128], BF16, kind="ExternalInput").ap()
    ohsk = nc.dram_tensor("ohsk", [NEX, 5, V, 128], BF16, kind="ExternalInput").ap()
    skipm = nc.dram_tensor("skipm", [128, FW], F32, kind="ExternalInput").ap()
    sel01 = nc.dram_tensor("sel01", [128, FW], F32, kind="ExternalInput").ap()
    selfin = nc.dram_tensor("selfin", [128, FW], F32, kind="ExternalInput").ap()
    negpen = nc.dram_tensor("negpen", [128, 1], F32, kind="ExternalInput").ap()
    shiftp = nc.dram_tensor("shiftp", [128, 128], F32, kind="ExternalInput").ap()
    ident = nc.dram_tensor("ident", [128, 128], F32, kind="ExternalInput").ap()
    onesb = nc.dram_tensor("onesb", [128, 128], BF16, kind="ExternalInput").ap()
    loss = nc.dram_tensor("loss", [1, NEX], F32, kind="ExternalOutput").ap()
    wdram = nc.dram_tensor("wdram", [NTB, NP, FW, TTILE], F32).ap()
    wskdram = nc.dram_tensor("wskdram", [NTB, NP, FW, TTILE], F32).ap()

    with TileContext(nc) as tc:
        # ---- persistent small tiles ----
        with tc.tile_pool(name="persist", bufs=1) as pp:
            shiftp_sb = pp.tile([128, 128], F32, tag="shiftp")
            nc.sync.dma_start(out=shiftp_sb[:], in_=shiftp[:])
            ident_sb = pp.tile([128, 128], F32, tag="ident")
            nc.sync.dma_start(out=ident_sb[:], in_=ident[:])
            ones_sb = pp.tile([128, 128], BF16, tag="ones")
            nc.sync.dma_start(out=ones_sb[:], in_=onesb[:])
            negpen_sb = pp.tile([128, 1], F32, tag="negpen")
            nc.sync.dma_start(out=negpen_sb[:], in_=negpen[:])
            skipm_sb = pp.tile([128, FW], F32, tag="skipm")
            nc.sync.dma_start(out=skipm_sb[:], in_=skipm[:])
            sel01_sb = pp.tile([128, FW], F32, tag="sel01")
            nc.sync.dma_start(out=sel01_sb[:], in_=sel01[:])
            selfin_sb = pp.tile([128, FW], F32, tag="selfin")
            nc.sync.dma_start(out=selfin_sb[:], in_=selfin[:])
            lnz = pp.tile([128, NEX], F32, tag="lnz")
            nc.gpsimd.memset(lnz[:], 0.0)
            lnr = pp.tile([128, 1], F32, tag="lnr")
            nc.gpsimd.memset(lnr[:], 0.0)
            rrat = pp.tile([128, 1], F32, tag="rrat")
            nc.gpsimd.memset(rrat[:], 1.0)

            # ================= phase A: build w streams ==================
            with tc.tile_pool(name="pa", bufs=2) as pa, \
                 tc.tile_pool(name="pap", bufs=2, space="PSUM") as pap, \
                 tc.tile_pool(name="pag", bufs=2, space="PSUM") as pag:
                for ex in range(NEX):
                    ut = pa.tile([128, T], BF16, tag="ut")
                    for it in range(NTB):
                        yt = pa.tile([128, V], F32, tag="yt")
                        nc.sync.dma_start(
                            out=yt[:], in_=y[ex, it * TTILE:(it + 1) * TTILE, :])
                        ytp = pap.tile([128, TTILE], F32, tag="ytp")
                        nc.tensor.transpose(ytp[:], yt[:], ident_sb[:])
                        nc.scalar.activation(
                            ut[:, it * TTILE:(it + 1) * TTILE], ytp[:],
                            AF.Exp, bias=negpen_sb[:], scale=1.0)
                    # lnZ accumulation
                    for tch in range(T // TCH):
                        zp = pag.tile([128, TCH], F32, tag="gmm")
                        nc.tensor.matmul(
                            zp[:], ones_sb[:],
                            ut[:, tch * TCH:(tch + 1) * TCH], start=True, stop=True)
                        lzt = pa.tile([128, TCH], F32, tag="lzt")
                        nc.scalar.activation(lzt[:], zp[:], AF.Ln)
                        lzr = pa.tile([128, 1], F32, tag="lzr")
                        nc.vector.tensor_reduce(lzr[:], lzt[:], AX.X, ALU.add)
                        nc.vector.tensor_add(
                            lnz[:, ex:ex + 1], lnz[:, ex:ex + 1], lzr[:])
                    # gathers
                    for j in range(5):
                        ohs = pa.tile([128, 128], BF16, tag="ohs")
                        nc.sync.dma_start(out=ohs[:], in_=oh[ex, j, :, :])
                        ohss = pa.tile([128, 128], BF16, tag="ohss")
                        nc.sync.dma_start(out=ohss[:], in_=ohsk[ex, j, :, :])
                        base = ex * NCH + j * 4
                        nch_here = 4 if j < 4 else 1
                        nrow = nch_here * FW
                        for tch in range(T // TCH):
                            for src_oh, dst_dram, tag in (
                                    (ohs, wdram, "gw"), (ohss, wskdram, "gs")):
                                gw = pag.tile([128, TCH], F32, tag="gmm")
                                nc.tensor.matmul(
                                    gw[:], src_oh[:],
                                    ut[:, tch * TCH:(tch + 1) * TCH],
                                    start=True, stop=True)
                                gsb = pa.tile([128, TCH], F32, tag=tag + "sb")
                                nc.scalar.copy(gsb[:], gw[:])
                                for ch in range(nch_here):
                                    dst = dst_dram[4 * tch:4 * tch + 4,
                                                   base + ch]
                                    dst = dst.rearrange("tb f ti -> f tb ti")
                                    src = gsb[ch * FW:(ch + 1) * FW, :]
                                    src = src.rearrange(
                                        "f (tb ti) -> f tb ti", ti=TTILE)
                                    nc.sync.dma_start(out=dst, in_=src)

            # ================= phase B: recursion ==================
            with tc.tile_pool(name="pb", bufs=2) as pb, \
                 tc.tile_pool(name="pbw", bufs=2) as pbw, \
                 tc.tile_pool(name="pbp", bufs=4, space="PSUM") as pbp, \
                 tc.tile_pool(name="pbr", bufs=2, space="PSUM") as pbr:
                ae = pb.tile([128, FW + 2], F32, tag="ae")
                nc.gpsimd.memset(ae[:], 0.0)
                w_sb = wsk_sb = None
                for tb in range(NTB):
                    w_new = pbw.tile([128, FW * TTILE], F32, tag="wsb")
                    nc.sync.dma_start(
                        out=w_new[0:NP, :],
                        in_=wdram[tb].rearrange("p f ti -> p (f ti)"))
                    wsk_new = pbw.tile([128, FW * TTILE], F32, tag="wsksb")
                    nc.sync.dma_start(
                        out=wsk_new[0:NP, :],
                        in_=wskdram[tb].rearrange("p f ti -> p (f ti)"))
                    w_sb, wsk_sb = w_new, wsk_new
                    w3 = w_sb[:].rearrange("p (f ti) -> p f ti", ti=TTILE)
                    wsk3 = wsk_sb[:].rearrange("p (f ti) -> p f ti", ti=TTILE)
                    for ti in range(TTILE):
                        t = tb * TTILE + ti
                        if t == 0:
                            # init: a = w[0] * sel01
                            nc.vector.tensor_mul(
                                ae[:, 2:2 + FW], w3[:, :, 0], sel01_sb[:])
                            continue
                        # boundary: prev chunk tail -> head cols, scaled by r
                        bnd = pbp.tile([128, 2], F32, tag="bnd")
                        nc.tensor.matmul(bnd[:], shiftp_sb[:],
                                         ae[:, FW:FW + 2], start=True, stop=True)
                        nc.scalar.mul(ae[:, 0:2], bnd[:], rrat[:])
                        t1 = pb.tile([128, FW], F32, tag="t1")
                        nc.vector.tensor_add(t1[:], ae[:, 1:1 + FW],
                                             ae[:, 2:2 + FW])
                        am2 = pb.tile([128, FW], F32, tag="am2")
                        nc.vector.tensor_mul(am2[:], ae[:, 0:FW], skipm_sb[:])
                        t3 = pb.tile([128, FW], F32, tag="t3")
                        nc.vector.tensor_add(t3[:], t1[:], am2[:])
                        nc.vector.tensor_mul(ae[:, 2:2 + FW], t3[:], w3[:, :, ti])
                        if t % K_RENORM == 0:
                            mx = pb.tile([128, 1], F32, tag="mx")
                            nc.vector.tensor_reduce(
                                mx[:], ae[:, 2:2 + FW], AX.X, ALU.max)
                            nc.vector.tensor_scalar_max(mx[:], mx[:], 1.0)
                            rz = pb.tile([128, 1], F32, tag="rz")
                            nc.vector.reciprocal(rz[:], mx[:])
                            nc.vector.tensor_scalar_mul(
                                ae[:, 2:2 + FW], ae[:, 2:2 + FW], rz[:])
                            lzz = pb.tile([128, 1], F32, tag="lzz")
                            nc.scalar.activation(lzz[:], mx[:], AF.Ln,
                                                 scale=float(2.0 ** -48))
                            nc.vector.scalar_tensor_tensor(
                                lnr[:], lzz[:], float(48 * np.log(2.0)),
                                lnr[:], ALU.add, ALU.add)
                            shl = pbr.tile([128, 1], F32, tag="shl")
                            nc.tensor.matmul(shl[:], shiftp_sb[:],
                                             lnr[:], start=True, stop=True)
                            dd = pb.tile([128, 1], F32, tag="dd")
                            nc.vector.tensor_tensor(
                                dd[:], shl[:], lnr[:], ALU.subtract)
                            nc.vector.tensor_scalar_min(dd[:], dd[:], CLIP)
                            nc.scalar.activation(rrat[:], dd[:], AF.Exp)

                # ============== phase C: extraction ==============
                bnd = pbp.tile([128, 2], F32, tag="bnd")
                nc.tensor.matmul(bnd[:], shiftp_sb[:], ae[:, FW:FW + 2],
                                 start=True, stop=True)
                nc.scalar.mul(ae[:, 0:2], bnd[:], rrat[:])
                ae2 = pb.tile([128, FW], F32, tag="ae2")
                nc.vector.tensor_add(ae2[:], ae[:, 1:1 + FW], ae[:, 2:2 + FW])
                exv = pb.tile([128, FW], F32, tag="exv")
                nc.vector.tensor_mul(exv[:], ae2[:], selfin_sb[:])
                exr = pb.tile([128, 1], F32, tag="exr")
                nc.vector.tensor_reduce(exr[:], exv[:], AX.X, ALU.add)
                lnex = pb.tile([128, 1], F32, tag="lnex")
                nc.scalar.activation(lnex[:], exr[:], AF.Ln,
                                     scale=float(2.0 ** -48))
                nc.vector.tensor_scalar_max(lnex[:], lnex[:], -1e30)
                contrib = pb.tile([128, 1], F32, tag="contrib")
                nc.gpsimd.memset(contrib[:], -1e30)
                nc.vector.scalar_tensor_tensor(
                    contrib[0:NP, :], lnex[0:NP, :], float(48 * np.log(2.0)),
                    lnr[0:NP, :], ALU.add, ALU.add)
                ctr = pbr.tile([1, 128], F32, tag="ctr")
                nc.tensor.transpose(ctr[:], contrib[:], ident_sb[:])
                mxc = pb.tile([1, NEX], F32, tag="mxc")
                nc.vector.tensor_reduce(
                    mxc[:], ctr[0:1, 0:NP].rearrange("p (e c) -> p e c", e=NEX),
                    AX.X, ALU.max)
                tmp = pb.tile([1, NEX], F32, tag="tmp")
                nc.vector.tensor_tensor(tmp[:], mxc[:], lnz[0:1, :],
                                        ALU.subtract)
                lossv = pb.tile([1, NEX], F32, tag="lossv")
                nc.vector.tensor_scalar(lossv[:], tmp[:], float(T * LNKAP),
                                        -1.0, ALU.subtract, ALU.mult)
                nc.sync.dma_start(out=loss[:], in_=lossv[:])

    nc.compile()
    _CACHED["nc"] = nc
    return nc


def _host_inputs(y_pred, y_true):
    """Per-core input maps."""
    maps = []
    shiftp = np.zeros((128, 128), np.float32)
    for p in range(NP - 1):
        if p % NCH != NCH - 1:
            shiftp[p, p + 1] = 1.0
    ident = np.eye(128, dtype=np.float32)
    onesb = np.ones((128, 128), ml_dtypes.bfloat16)
    negpen = np.zeros((128, 1), np.float32)
    negpen[:V, 0] = -PEN
    sel01 = np.zeros((128, FW), np.float32)
    for ex in range(NEX):
        sel01[ex * NCH, 0] = 1.0
        sel01[ex * NCH, 1] = 1.0
    for c in range(NCORE):
        exs = slice(c * NEX, (c + 1) * NEX)
        yc = np.ascontiguousarray(y_pred[exs]).astype(np.float32)
        ytc = y_true[exs]
        oh = np.zeros((NEX, 5, V, 128), np.float32)
        ohsk = np.zeros((NEX, 5, V, 128), np.float32)
        skipm = np.zeros((128, FW), np.float32)
        selfin = np.zeros((128, FW), np.float32)
        for ex in range(NEX):
            lab = ytc[ex]
            length = int((lab != 0).sum())
            ext = np.zeros(SP, np.int64)
            ext[1:2 * L + 1:2] = lab
            skip = np.zeros(SP, np.float32)
            for s in range(2, S):
                if ext[s] != 0 and ext[s] != ext[s - 2]:
                    skip[s] = 1.0
            skipm[ex * NCH:(ex + 1) * NCH, :] = skip.reshape(NCH, FW)
            for s in range(S):
                j, p = divmod(s, 128)
                oh[ex, j, ext[s], p] = KAPPA
                if skip[s]:
                    ohsk[ex, j, ext[s], p] = KAPPA
            i = 2 * length
            selfin[ex * NCH + i // FW, i % FW] = 1.0
        maps.append({
            "y": yc,
            "oh": oh.astype(ml_dtypes.bfloat16),
            "ohsk": ohsk.astype(ml_dtypes.bfloat16),
            "skipm": skipm, "sel01": sel01, "selfin": selfin,
            "negpen": negpen, "shiftp": shiftp, "ident": ident,
            "onesb": onesb,
        })
    return maps


def kernel(y_pred, y_true):
    y_pred = np.asarray(y_pred, dtype=np.float32)
    y_true = np.asarray(y_true, dtype=np.int32)
    nc = _build_program()
    maps = _host_inputs(y_pred, y_true)
    res = run_bass_kernel_spmd(nc, maps, core_ids=list(range(NCORE)))
    losses = np.concatenate([res.results[c]["loss"][0] for c in range(NCORE)])
    return np.float32(np.mean(losses) + 1e-7)

